# revision 4
# baseline (speedup 1.0000x reference)
"""Trainium2 Bass kernel for AdaptivePrototypeContrastiveLoss.

Strategy
--------
Host (cheap, O(N*D) bookkeeping):
  * closed-form momentum EMA + LAPACK QR -> new prototypes  [7,256]
  * row-normalize feats, stable-sort rows by label, append 7 per-class
    sum columns (Y) so the tiny "positive" matmul rides the main sweep
  * precompute per-row constants (alpha/beta/valid/onehot)

Device (8 NeuronCores, SPMD, no collectives; all O(N^2) work):
  * row-shard: each core owns 8 row-tiles of 128 rows (64 tiles cover
    rows 0..8191); the last 7 rows' column sweep (row-tile 64) is
    split column-wise across all 8 cores as class-pure 512-chunks
  * per row-tile: G = rows @ feats^T via PE (fp8-e4m3 DoubleRow, f32
    PSUM, K=256 per instruction, 512-col chunks in 2048-col supertiles)
  * ACT computes exp(A*sim + BIAS) from PSUM into a bf16 SBUF scratch
    row (one 2048-wide ACT per supertile + a 16-wide tail that also
    carries the Y columns)
  * DVE computes the per-class segment sums with tensor_scalar+accum_out
    (InstTensorScalarPtr, all-bf16 SBUF operands -> 4x_2p DVE mode);
    columns are label-sorted so class segments are contiguous and
    identical on all cores -> the graph stays SPMD-uniform
  * neg_i = total - own-class (selected via shipped onehot); the
    global max subtraction is replaced by the constant M0=12.5 (the
    max only enters through ~1e-8-scale eps terms, verified offline)
  * per-core output: 128-partition partial sums of thresholded loss
    + 3 column-chunk exp sums for the shared row-tile 64
Host: combine 8x[128,8] partials -> scalar.
"""

import ml_dtypes
import numpy as np

import concourse.bass as bass
import concourse.tile as tile
from concourse import mybir
from concourse.bass_utils import run_bass_kernel_spmd

# ---- problem constants (hardcoded per spec) ----
TEMP = 0.08
EPS = 1e-8
GAMMA = 0.99
BETA = 0.5 * (1.0 - GAMMA)
B, D, C = 8192, 256, 7
N = B + C                      # 8199 rows/cols of the score matrix
NCORES = 8
NT = 8                         # full row-tiles per core (8*8*128 = 8192)
ROWS_PER_CORE = NT * 128       # 1024
NF = 8208                      # cols: 8199 data + 1 pad + 8 Y
YOFF = 8200                    # Y columns at 8200..8208
SUPER = 2048                   # psum supertile width (4 banks)
NSUP = 4                       # full supertiles cover cols 0..8192
SCRW = 8200                    # bf16 scratch row width (data + pad col)
T8W = 1536                     # per-core share of row-tile 64's columns
M0 = 12.5                      # constant stand-in for the global max
A_SCALE = 0.5 / float(np.float32(TEMP))
BIAS = (0.5 + EPS) / float(np.float32(TEMP)) - M0

F32 = mybir.dt.float32
BF16 = mybir.dt.bfloat16
FP8 = mybir.dt.float8e4
FP8NP = mybir.dt.np(mybir.dt.float8e4)
ALU = mybir.AluOpType
ACTF = mybir.ActivationFunctionType


def _split_multi_waits(nc):
    """This container's walrus accepts only ONE sync wait per instruction;
    split extra waits into standalone single-wait EventSemaphore insts."""
    n_new = 0
    for func in nc.m.functions:
        for blk in func.blocks:
            new_insts = []
            for inst in blk.instructions:
                si = getattr(inst, "sync_info", None)
                waits = list(si.on_wait) if si and si.on_wait else []
                if len(waits) > 1:
                    for i, w in enumerate(waits[:-1]):
                        n_new += 1
                        ev = mybir.InstEventSemaphore(
                            name=f"{inst.name}-wsplit{i}",
                            engine=inst.engine,
                            ins=[],
                            outs=[],
                            sync_info=mybir.SyncInfo(on_wait=[w], on_update=[]),
                            bass_nofuse=True,
                        )
                        new_insts.append(ev)
                    si.on_wait = [waits[-1]]
                new_insts.append(inst)
            blk.instructions = new_insts
    return n_new


def _host_prep(features, labels, prototypes, momentums):
    features = np.asarray(features, dtype=np.float32)
    labels = np.asarray(labels).astype(np.int64)
    prototypes = np.asarray(prototypes, dtype=np.float32)
    momentums = np.asarray(momentums, dtype=np.float32)

    # ---- prototype update: closed form of the sequential EMA scan ----
    counts_feat = np.bincount(labels, minlength=C)
    rank = np.zeros(B, dtype=np.int64)
    seen = np.zeros(C, dtype=np.int64)
    for i, l in enumerate(labels):
        rank[i] = seen[l]
        seen[l] += 1
    w = BETA * (GAMMA ** (counts_feat[labels] - 1 - rank).astype(np.float64))
    S = np.zeros((C, B))
    S[labels, np.arange(B)] = w
    m_final = S @ features.astype(np.float64)
    wsum = np.bincount(labels, weights=w, minlength=C)
    m_final -= wsum[:, None] * prototypes.astype(np.float64)
    m_final += (GAMMA ** counts_feat.astype(np.float64))[:, None] * momentums.astype(
        np.float64
    )
    target = prototypes.astype(np.float64) + m_final
    q, _ = np.linalg.qr(target.T.astype(np.float32))
    new_protos = q.T.astype(np.float32)

    # ---- normalized, label-sorted gram operands ----
    feats = np.concatenate([features, new_protos], 0)
    labs = np.concatenate([labels, np.arange(C, dtype=np.int64)])
    nrm = np.linalg.norm(feats.astype(np.float64), axis=-1)
    fhat = feats.astype(np.float64) / nrm[:, None]
    perm = np.argsort(labs, kind="stable")
    fs = fhat[perm]
    ls = labs[perm]
    counts_all = np.bincount(ls, minlength=C)          # includes protos
    bounds = np.concatenate([[0], np.cumsum(counts_all)])  # class col ranges
    assert all(bounds[c + 1] - bounds[c] >= 2 for c in range(C))

    fs32 = fs.astype(np.float32)
    Y = np.zeros((D, 8), dtype=np.float64)
    for c in range(C):
        Y[:, c] = fs[bounds[c]:bounds[c + 1]].sum(0)

    ftpad = np.zeros((NF, D), dtype=np.float32)
    ftpad[:N] = fs32
    ftpad[YOFF:YOFF + 8] = Y.T.astype(np.float32)
    ft = np.ascontiguousarray(
        ftpad.T.reshape(2, 128, NF).transpose(1, 0, 2)
    ).astype(FP8NP)  # [partition, k-half, col] for DoubleRow

    # ---- per-row constants ----
    cnt = counts_all[ls] - 1
    selfsim = (fs32.astype(np.float64) ** 2).sum(1)
    inv = 1.0 / (cnt.astype(np.float64) + EPS)
    alpha_all = A_SCALE * inv
    beta_all = (-A_SCALE * selfsim + BIAS * cnt) * inv

    # ---- shared row-tile 64 (last 7 rows), column-split across cores ----
    t8block = np.zeros((128, D), dtype=np.float32)
    t8block[:N - B] = fs32[B:N]
    t8rows = np.ascontiguousarray(
        t8block.T.reshape(2, 128, 128).transpose(1, 0, 2)
    ).astype(FP8NP)
    chunk_cols = []  # class-pure 512-col chunks (global col indices)
    chunk_cls = []
    for c in range(C):
        cols = np.arange(bounds[c], bounds[c + 1])
        for o in range(0, len(cols), 512):
            chunk_cols.append(cols[o:o + 512])
            chunk_cls.append(c)
    n_cpc = T8W // 512  # chunks per core
    while len(chunk_cols) < NCORES * n_cpc:
        chunk_cols.append(np.zeros(0, dtype=np.int64))
        chunk_cls.append(-1)
    t8meta = []  # (class, n_pad) per chunk for the host-side combine
    ft_np = np.asarray(ft)
    t8cols_per_core = []
    for core in range(NCORES):
        arr = np.zeros((128, 2, T8W), dtype=FP8NP)
        for j in range(n_cpc):
            ci = core * n_cpc + j
            cols = chunk_cols[ci]
            arr[:, :, j * 512:j * 512 + len(cols)] = ft_np[:, :, cols]
            t8meta.append((chunk_cls[ci], 512 - len(cols)))
        t8cols_per_core.append(arr)

    per_core = []
    for core in range(NCORES):
        base = core * ROWS_PER_CORE
        rows_kt = np.ascontiguousarray(
            ftpad[base:base + ROWS_PER_CORE].T.reshape(2, 128, ROWS_PER_CORE)
            .transpose(1, 0, 2)
        ).astype(FP8NP)

        onehot = np.zeros((NT, 128, 8), dtype=np.float32)
        rowmeta = np.zeros((128, 3 * NT), dtype=np.float32)  # alpha|beta|valid
        for t in range(NT):
            for p in range(128):
                g = base + t * 128 + p
                onehot[t, p, ls[g]] = 1.0
                rowmeta[p, t] = alpha_all[g]
                rowmeta[p, NT + t] = beta_all[g]
                rowmeta[p, 2 * NT + t] = 1.0
        per_core.append(
            {
                "ft": ft,
                "rows": rows_kt,
                "onehot": onehot,
                "rowmeta": rowmeta,
                "t8rows": t8rows,
                "t8cols": t8cols_per_core[core],
            }
        )

    host = {
        "ls": ls, "bounds": bounds, "counts_all": counts_all, "fs": fs,
        "Y": Y, "t8meta": t8meta, "selfsim": selfsim, "cnt": cnt,
    }
    return per_core, host


def _build_graph(bounds):
    nc = bass.Bass()
    ft_d = nc.declare_dram_parameter("ft", [128, 2, NF], FP8, isOutput=False)
    rows_d = nc.declare_dram_parameter(
        "rows", [128, 2, ROWS_PER_CORE], FP8, isOutput=False
    )
    oh_d = nc.declare_dram_parameter("onehot", [NT, 128, 8], F32, isOutput=False)
    meta_d = nc.declare_dram_parameter("rowmeta", [128, 3 * NT], F32, isOutput=False)
    t8r_d = nc.declare_dram_parameter("t8rows", [128, 2, 128], FP8, isOutput=False)
    t8c_d = nc.declare_dram_parameter("t8cols", [128, 2, T8W], FP8, isOutput=False)
    out_d = nc.declare_dram_parameter("out", [128, 8], F32, isOutput=True)

    with tile.TileContext(nc) as tc:
        with (
            tc.tile_pool(name="persist", bufs=1) as persist,
            tc.tile_pool(name="ps", bufs=2, space="PSUM") as psA,
            tc.tile_pool(name="small", bufs=4) as small,
        ):
            # --- resident inputs ---
            rows_sb = persist.tile([128, 2, ROWS_PER_CORE], FP8, tag="rows")
            nc.sync.dma_start(out=rows_sb[:], in_=rows_d[:])
            t8r_sb = persist.tile([128, 2, 128], FP8, tag="t8r")
            nc.sync.dma_start(out=t8r_sb[:], in_=t8r_d[:])
            t8c_sb = persist.tile([128, 2, T8W], FP8, tag="t8c")
            nc.sync.dma_start(out=t8c_sb[:], in_=t8c_d[:])
            meta_sb = persist.tile([128, 3 * NT], F32, tag="meta")
            nc.sync.dma_start(out=meta_sb[:], in_=meta_d[:])
            oh_sb = persist.tile([128, NT, 8], F32, tag="oh")
            for t in range(NT):
                nc.sync.dma_start(out=oh_sb[:, t, :], in_=oh_d[t])
            ft_sb = persist.tile([128, 2, NF], FP8, tag="ft")
            for s in range(NSUP):
                lo = s * SUPER
                hi = NF if s == NSUP - 1 else lo + SUPER
                nc.sync.dma_start(out=ft_sb[:, :, lo:hi], in_=ft_d[:, :, lo:hi])

            slots_all = persist.tile([128, NT, 8], F32, tag="slots_all")
            nc.vector.memset(slots_all[:], 0.0)
            praw = persist.tile([128, NT, 8], F32, tag="praw")
            bias_exp = persist.tile([128, 1], F32, tag="bias_exp")
            nc.vector.memset(bias_exp[:], float(BIAS))
            bias_ln = persist.tile([128, 1], F32, tag="bias_ln")
            nc.vector.memset(bias_ln[:], float(EPS))
            out_t = persist.tile([128, 8], F32, tag="out")

            scr_a = persist.tile([128, SCRW], BF16, tag="scr0")
            scr_b = persist.tile([128, SCRW], BF16, tag="scr1")
            scrs = [scr_a, scr_b]
            garb = persist.tile([128, 1536], BF16, tag="garb")
            scr8 = persist.tile([128, T8W], BF16, tag="scr8")

            # --- main loop over row-tiles ---
            for t in range(NT):
                scr = scrs[t % 2]
                for s in range(NSUP):
                    lo = s * SUPER
                    ps = psA.tile([128, SUPER], F32, tag="ps")
                    for j in range(SUPER // 512):
                        nc.tensor.matmul(
                            ps[:, j * 512:(j + 1) * 512],
                            lhsT=rows_sb[:, :, t * 128:(t + 1) * 128],
                            rhs=ft_sb[:, :, lo + j * 512:lo + (j + 1) * 512],
                            start=True,
                            stop=True,
                            perf_mode=mybir.MatmulPerfMode.DoubleRow,
                        )
                    nc.scalar.activation(
                        scr[:, lo:lo + SUPER],
                        ps[:],
                        ACTF.Exp,
                        bias=bias_exp[:],
                        scale=float(A_SCALE),
                    )
                # tail: data cols 8192..8199 + Y cols, one 16-wide matmul
                pst = psA.tile([128, SUPER], F32, tag="ps")
                nc.tensor.matmul(
                    pst[:, 0:16],
                    lhsT=rows_sb[:, :, t * 128:(t + 1) * 128],
                    rhs=ft_sb[:, :, 8192:8208],
                    start=True,
                    stop=True,
                    perf_mode=mybir.MatmulPerfMode.DoubleRow,
                )
                nc.scalar.activation(
                    scr[:, 8192:8200],
                    pst[:, 0:8],
                    ACTF.Exp,
                    bias=bias_exp[:],
                    scale=float(A_SCALE),
                )
                nc.vector.tensor_copy(praw[:, t, :], pst[:, 8:16])
                # per-class segment sums via tensor_scalar accumulate
                # (all-bf16 SBUF operands -> 4x DVE mode)
                for c in range(C):
                    lo_c = int(bounds[c])
                    hi_c = int(bounds[c + 1])
                    nc.vector.tensor_scalar(
                        garb[:, 0:hi_c - lo_c],
                        scr[:, lo_c:hi_c],
                        1.0,
                        None,
                        ALU.mult,
                        op1=ALU.add,
                        accum_out=slots_all[:, t, c:c + 1],
                    )

            # --- shared row-tile 64: this core's column slice ---
            ps8 = psA.tile([128, SUPER], F32, tag="ps")
            for j in range(T8W // 512):
                nc.tensor.matmul(
                    ps8[:, j * 512:(j + 1) * 512],
                    lhsT=t8r_sb[:],
                    rhs=t8c_sb[:, :, j * 512:(j + 1) * 512],
                    start=True,
                    stop=True,
                    perf_mode=mybir.MatmulPerfMode.DoubleRow,
                )
            nc.scalar.activation(
                scr8[:], ps8[:, 0:T8W], ACTF.Exp,
                bias=bias_exp[:], scale=float(A_SCALE),
            )
            for j in range(T8W // 512):
                nc.vector.tensor_scalar(
                    garb[:, 0:512],
                    scr8[:, j * 512:(j + 1) * 512],
                    1.0,
                    None,
                    ALU.mult,
                    op1=ALU.add,
                    accum_out=out_t[:, 2 + j:3 + j],
                )

            # --- epilogue: loss, threshold, partial sums ---
            stot9 = small.tile([128, NT], F32, tag="stot9")
            nc.vector.reduce_sum(stot9[:], slots_all[:], mybir.AxisListType.X)
            owns_all = small.tile([128, NT, 8], F32, tag="owns_all")
            nc.vector.tensor_tensor(
                out=owns_all[:], in0=slots_all[:], in1=oh_sb[:], op=ALU.mult
            )
            sown9 = small.tile([128, NT], F32, tag="sown9")
            nc.vector.reduce_sum(sown9[:], owns_all[:], mybir.AxisListType.X)
            negsum9 = small.tile([128, NT], F32, tag="negsum9")
            nc.vector.tensor_tensor(
                out=negsum9[:], in0=stot9[:], in1=sown9[:], op=ALU.subtract
            )
            pm_all = small.tile([128, NT, 8], F32, tag="pm_all")
            nc.vector.tensor_tensor(
                out=pm_all[:], in0=praw[:], in1=oh_sb[:], op=ALU.mult
            )
            possel9 = small.tile([128, NT], F32, tag="possel9")
            nc.vector.reduce_sum(possel9[:], pm_all[:], mybir.AxisListType.X)

            alpha9 = meta_sb[:, 0:NT]
            beta9 = meta_sb[:, NT:2 * NT]
            valid9 = meta_sb[:, 2 * NT:3 * NT]
            pos9 = small.tile([128, NT], F32, tag="pos9")
            nc.vector.tensor_tensor(
                out=pos9[:], in0=possel9[:], in1=alpha9, op=ALU.mult
            )
            nc.vector.tensor_tensor(out=pos9[:], in0=pos9[:], in1=beta9, op=ALU.add)
            neg9 = small.tile([128, NT], F32, tag="neg9")
            nc.scalar.activation(
                neg9[:], negsum9[:], ACTF.Ln, bias=bias_ln[:], scale=1.0
            )
            loss9 = small.tile([128, NT], F32, tag="loss9")
            nc.vector.tensor_tensor(
                out=loss9[:], in0=neg9[:], in1=pos9[:], op=ALU.subtract
            )
            gt9 = small.tile([128, NT], F32, tag="gt9")
            nc.vector.tensor_scalar(
                out=gt9[:], in0=loss9[:], scalar1=0.0, scalar2=None, op0=ALU.is_gt
            )
            nc.vector.tensor_tensor(out=gt9[:], in0=gt9[:], in1=valid9, op=ALU.mult)
            contrib9 = small.tile([128, NT], F32, tag="contrib9")
            nc.vector.tensor_tensor(
                out=contrib9[:], in0=loss9[:], in1=gt9[:], op=ALU.mult
            )
            nc.vector.reduce_sum(out_t[:, 0:1], contrib9[:], mybir.AxisListType.X)
            nc.vector.reduce_sum(out_t[:, 1:2], gt9[:], mybir.AxisListType.X)
            nc.sync.dma_start(out=out_d[:], in_=out_t[:])
    return nc


def _combine(results, host):
    """Host-side unshard: merge per-core partials + finish row-tile 64."""
    ls = host["ls"]
    fs, Y = host["fs"], host["Y"]
    loss_sum = 0.0
    cnt_sum = 0.0
    for r in results:
        o = np.asarray(r["out"], dtype=np.float64)
        loss_sum += o[:, 0].sum()
        cnt_sum += o[:, 1].sum()

    # row-tile 64: rows 8192..8198 — class sums from per-core chunk sums
    pad_exp = float(
        ml_dtypes.bfloat16(np.exp(np.float32(BIAS)))
    )  # a zero pad column's exp as the device computes it
    n7 = N - B  # 7
    n_cpc = T8W // 512
    classsum = np.zeros((n7, C), dtype=np.float64)
    for core in range(NCORES):
        o = np.asarray(results[core]["out"], dtype=np.float64)
        for j in range(n_cpc):
            cls, n_pad = host["t8meta"][core * n_cpc + j]
            if cls < 0:
                continue
            classsum[:, cls] += o[:n7, 2 + j] - n_pad * pad_exp
    stot = classsum.sum(1)
    rows_ls = ls[B:N]
    sown = classsum[np.arange(n7), rows_ls]
    neg = np.log(stot - sown + EPS)
    pos_sel = np.einsum("id,di->i", fs[B:N], Y[:, rows_ls])
    selfsim = host["selfsim"][B:N]
    cnt = host["cnt"][B:N]
    pos = (A_SCALE * (pos_sel - selfsim) + BIAS * cnt) / (cnt + EPS)
    loss64 = -pos + neg
    m = loss64 > 0
    loss_sum += loss64[m].sum()
    cnt_sum += m.sum()

    val = loss_sum / max(cnt_sum, 1.0) if cnt_sum > 0 else 0.0
    return np.float32(val)


def _run(features, labels, prototypes, momentums, trace=False, trace_kwargs=None):
    per_core, host = _host_prep(features, labels, prototypes, momentums)
    nc = _build_graph(host["bounds"])
    _split_multi_waits(nc)
    in_maps = [per_core[i] for i in range(NCORES)]
    kw = {}
    if trace:
        kw = dict(trace=True, trace_cores=list(range(NCORES)))
        if trace_kwargs:
            kw["trace_kwargs"] = trace_kwargs
    res = run_bass_kernel_spmd(nc, in_maps, core_ids=list(range(NCORES)), **kw)
    return _combine(res.results, host), res


def kernel(features, labels, prototypes, momentums):
    val, _ = _run(features, labels, prototypes, momentums)
    return np.array(val, dtype=np.float32)


# revision 5
# speedup vs baseline: 1.3077x; 1.3077x over previous
"""Trainium2 Bass kernel for AdaptivePrototypeContrastiveLoss.

Strategy
--------
Host (cheap, O(N*D) bookkeeping):
  * closed-form momentum EMA + LAPACK QR -> new prototypes  [7,256]
  * row-normalize feats, stable-sort rows by label; scatter the sorted
    columns into 7 uniform class segments of 1280 (zero-padded) and
    append 7 per-class sum columns (Y) so the tiny "positive" matmul
    rides the main sweep
  * precompute per-row constants (alpha/beta/valid/onehot)

Device (8 NeuronCores, SPMD, no collectives; all O(N^2) work):
  * row-shard: each core owns 8 row-tiles of 128 rows (64 tiles cover
    rows 0..8191); the last 7 rows' column sweep (row-tile 64) is
    split column-wise across all 8 cores as class-pure 512-chunks
  * per row-tile: G = rows @ feats^T via PE (fp8-e4m3 DoubleRow, f32
    PSUM, K=256 per instruction, 512-col chunks in 2048-col supertiles)
  * ACT computes exp(A*sim + BIAS) from PSUM into a flat bf16 SBUF
    scratch row (2048-wide ACTs + a 768-wide tail)
  * DVE reduces each class segment with a log2 fold tree: 4 batched
    bf16 tensor_tensor adds on a [128,7,*] strided view (2x DVE mode),
    then one batched TENSOR_REDUCE -> [128,7] class sums.  Uniform
    1280-wide segments keep every AP static and SPMD-uniform.
  * neg_i = total - own-class (selected via shipped onehot); the
    global max subtraction is replaced by the constant M0=12.5 (the
    max only enters through ~1e-8-scale eps terms, verified offline)
  * per-core output: 128-partition partial sums of thresholded loss
    + 3 column-chunk exp sums for the shared row-tile 64
Host: combine 8x[128,8] partials -> scalar.
"""

import ml_dtypes
import numpy as np

import concourse.bass as bass
import concourse.tile as tile
from concourse import mybir
from concourse.bass_utils import run_bass_kernel_spmd

# ---- problem constants (hardcoded per spec) ----
TEMP = 0.08
EPS = 1e-8
GAMMA = 0.99
BETA = 0.5 * (1.0 - GAMMA)
B, D, C = 8192, 256, 7
N = B + C                      # 8199 rows/cols of the score matrix
NCORES = 8
NT = 8                         # full row-tiles per core (8*8*128 = 8192)
ROWS_PER_CORE = NT * 128       # 1024
W = 1280                       # uniform padded class-segment width
NPAD = C * W                   # 8960 data columns
YOFF = NPAD                    # Y columns at 8960..8968
NF = 8976                      # total ft columns (16-aligned)
SUPER = 2048                   # psum supertile width (4 banks)
NSUP = 4                       # full supertiles cover cols 0..8192
TAILW = NPAD - NSUP * SUPER    # 768 data cols in the tail supertile
T8W = 1536                     # per-core share of row-tile 64's columns
M0 = 12.5                      # constant stand-in for the global max
A_SCALE = 0.5 / float(np.float32(TEMP))
BIAS = (0.5 + EPS) / float(np.float32(TEMP)) - M0

F32 = mybir.dt.float32
BF16 = mybir.dt.bfloat16
FP8 = mybir.dt.float8e4
FP8NP = mybir.dt.np(mybir.dt.float8e4)
ALU = mybir.AluOpType
ACTF = mybir.ActivationFunctionType


def _split_multi_waits(nc):
    """This container's walrus accepts only ONE sync wait per instruction;
    split extra waits into standalone single-wait EventSemaphore insts."""
    n_new = 0
    for func in nc.m.functions:
        for blk in func.blocks:
            new_insts = []
            for inst in blk.instructions:
                si = getattr(inst, "sync_info", None)
                waits = list(si.on_wait) if si and si.on_wait else []
                if len(waits) > 1:
                    for i, w in enumerate(waits[:-1]):
                        n_new += 1
                        ev = mybir.InstEventSemaphore(
                            name=f"{inst.name}-wsplit{i}",
                            engine=inst.engine,
                            ins=[],
                            outs=[],
                            sync_info=mybir.SyncInfo(on_wait=[w], on_update=[]),
                            bass_nofuse=True,
                        )
                        new_insts.append(ev)
                    si.on_wait = [waits[-1]]
                new_insts.append(inst)
            blk.instructions = new_insts
    return n_new


def _host_prep(features, labels, prototypes, momentums):
    features = np.asarray(features, dtype=np.float32)
    labels = np.asarray(labels).astype(np.int64)
    prototypes = np.asarray(prototypes, dtype=np.float32)
    momentums = np.asarray(momentums, dtype=np.float32)

    # ---- prototype update: closed form of the sequential EMA scan ----
    counts_feat = np.bincount(labels, minlength=C)
    rank = np.zeros(B, dtype=np.int64)
    seen = np.zeros(C, dtype=np.int64)
    for i, l in enumerate(labels):
        rank[i] = seen[l]
        seen[l] += 1
    w = BETA * (GAMMA ** (counts_feat[labels] - 1 - rank).astype(np.float64))
    S = np.zeros((C, B))
    S[labels, np.arange(B)] = w
    m_final = S @ features.astype(np.float64)
    wsum = np.bincount(labels, weights=w, minlength=C)
    m_final -= wsum[:, None] * prototypes.astype(np.float64)
    m_final += (GAMMA ** counts_feat.astype(np.float64))[:, None] * momentums.astype(
        np.float64
    )
    target = prototypes.astype(np.float64) + m_final
    q, _ = np.linalg.qr(target.T.astype(np.float32))
    new_protos = q.T.astype(np.float32)

    # ---- normalized, label-sorted gram operands ----
    feats = np.concatenate([features, new_protos], 0)
    labs = np.concatenate([labels, np.arange(C, dtype=np.int64)])
    nrm = np.linalg.norm(feats.astype(np.float64), axis=-1)
    fhat = feats.astype(np.float64) / nrm[:, None]
    perm = np.argsort(labs, kind="stable")
    fs = fhat[perm]
    ls = labs[perm]
    counts_all = np.bincount(ls, minlength=C)          # includes protos
    bounds = np.concatenate([[0], np.cumsum(counts_all)])  # class col ranges
    assert counts_all.max() <= W, counts_all

    fs32 = fs.astype(np.float32)
    Y = np.zeros((D, 8), dtype=np.float64)
    for c in range(C):
        Y[:, c] = fs[bounds[c]:bounds[c + 1]].sum(0)

    # columns scattered into uniform 1280-wide zero-padded class segments
    ftcols = np.zeros((NF, D), dtype=np.float32)
    for c in range(C):
        cnt = int(counts_all[c])
        ftcols[c * W:c * W + cnt] = fs32[bounds[c]:bounds[c + 1]]
    ftcols[YOFF:YOFF + 8] = Y.T.astype(np.float32)
    ft = np.ascontiguousarray(
        ftcols.T.reshape(2, 128, NF).transpose(1, 0, 2)
    ).astype(FP8NP)  # [partition, k-half, col] for DoubleRow

    # ---- per-row constants ----
    cnt = counts_all[ls] - 1
    selfsim = (fs32.astype(np.float64) ** 2).sum(1)
    inv = 1.0 / (cnt.astype(np.float64) + EPS)
    alpha_all = A_SCALE * inv
    beta_all = (-A_SCALE * selfsim + BIAS * cnt) * inv

    # ---- shared row-tile 64 (last 7 rows), column-split across cores ----
    t8block = np.zeros((128, D), dtype=np.float32)
    t8block[:N - B] = fs32[B:N]
    t8rows = np.ascontiguousarray(
        t8block.T.reshape(2, 128, 128).transpose(1, 0, 2)
    ).astype(FP8NP)
    chunk_cols = []  # class-pure 512-col chunks (padded-layout col indices)
    chunk_cls = []
    for c in range(C):
        cols = np.arange(c * W, c * W + int(counts_all[c]))
        for o in range(0, len(cols), 512):
            chunk_cols.append(cols[o:o + 512])
            chunk_cls.append(c)
    n_cpc = T8W // 512  # chunks per core
    while len(chunk_cols) < NCORES * n_cpc:
        chunk_cols.append(np.zeros(0, dtype=np.int64))
        chunk_cls.append(-1)
    t8meta = []  # (class, n_pad) per chunk for the host-side combine
    ft_np = np.asarray(ft)
    t8cols_per_core = []
    for core in range(NCORES):
        arr = np.zeros((128, 2, T8W), dtype=FP8NP)
        for j in range(n_cpc):
            ci = core * n_cpc + j
            cols = chunk_cols[ci]
            arr[:, :, j * 512:j * 512 + len(cols)] = ft_np[:, :, cols]
            t8meta.append((chunk_cls[ci], 512 - len(cols)))
        t8cols_per_core.append(arr)

    per_core = []
    for core in range(NCORES):
        base = core * ROWS_PER_CORE
        rows_kt = np.ascontiguousarray(
            fs32[base:base + ROWS_PER_CORE].T.reshape(2, 128, ROWS_PER_CORE)
            .transpose(1, 0, 2)
        ).astype(FP8NP)

        onehot = np.zeros((NT, 128, 8), dtype=np.float32)
        rowmeta = np.zeros((128, 3 * NT), dtype=np.float32)  # alpha|beta|valid
        for t in range(NT):
            for p in range(128):
                g = base + t * 128 + p
                onehot[t, p, ls[g]] = 1.0
                rowmeta[p, t] = alpha_all[g]
                rowmeta[p, NT + t] = beta_all[g]
                rowmeta[p, 2 * NT + t] = 1.0
        per_core.append(
            {
                "ft": ft,
                "rows": rows_kt,
                "onehot": onehot,
                "rowmeta": rowmeta,
                "t8rows": t8rows,
                "t8cols": t8cols_per_core[core],
            }
        )

    host = {
        "ls": ls, "bounds": bounds, "counts_all": counts_all, "fs": fs,
        "Y": Y, "t8meta": t8meta, "selfsim": selfsim, "cnt": cnt,
    }
    return per_core, host


def _build_graph():
    nc = bass.Bass()
    ft_d = nc.declare_dram_parameter("ft", [128, 2, NF], FP8, isOutput=False)
    rows_d = nc.declare_dram_parameter(
        "rows", [128, 2, ROWS_PER_CORE], FP8, isOutput=False
    )
    oh_d = nc.declare_dram_parameter("onehot", [NT, 128, 8], F32, isOutput=False)
    meta_d = nc.declare_dram_parameter("rowmeta", [128, 3 * NT], F32, isOutput=False)
    t8r_d = nc.declare_dram_parameter("t8rows", [128, 2, 128], FP8, isOutput=False)
    t8c_d = nc.declare_dram_parameter("t8cols", [128, 2, T8W], FP8, isOutput=False)
    out_d = nc.declare_dram_parameter("out", [128, 8], F32, isOutput=True)

    with tile.TileContext(nc) as tc:
        with (
            tc.tile_pool(name="persist", bufs=1) as persist,
            tc.tile_pool(name="ps", bufs=2, space="PSUM") as psA,
            tc.tile_pool(name="small", bufs=4) as small,
        ):
            # --- resident inputs ---
            rows_sb = persist.tile([128, 2, ROWS_PER_CORE], FP8, tag="rows")
            nc.sync.dma_start(out=rows_sb[:], in_=rows_d[:])
            t8r_sb = persist.tile([128, 2, 128], FP8, tag="t8r")
            nc.sync.dma_start(out=t8r_sb[:], in_=t8r_d[:])
            t8c_sb = persist.tile([128, 2, T8W], FP8, tag="t8c")
            nc.sync.dma_start(out=t8c_sb[:], in_=t8c_d[:])
            meta_sb = persist.tile([128, 3 * NT], F32, tag="meta")
            nc.sync.dma_start(out=meta_sb[:], in_=meta_d[:])
            oh_sb = persist.tile([128, NT, 8], F32, tag="oh")
            for t in range(NT):
                nc.sync.dma_start(out=oh_sb[:, t, :], in_=oh_d[t])
            ft_sb = persist.tile([128, 2, NF], FP8, tag="ft")
            for s in range(NSUP + 1):
                lo = s * SUPER
                hi = NF if s == NSUP else lo + SUPER
                nc.sync.dma_start(out=ft_sb[:, :, lo:hi], in_=ft_d[:, :, lo:hi])

            slots_all = persist.tile([128, NT, 8], F32, tag="slots_all")
            nc.vector.memset(slots_all[:], 0.0)
            praw = persist.tile([128, NT, 8], F32, tag="praw")
            bias_exp = persist.tile([128, 1], F32, tag="bias_exp")
            nc.vector.memset(bias_exp[:], float(BIAS))
            bias_ln = persist.tile([128, 1], F32, tag="bias_ln")
            nc.vector.memset(bias_ln[:], float(EPS))
            out_t = persist.tile([128, 8], F32, tag="out")

            scr_a = persist.tile([128, NPAD], BF16, tag="scr0")
            scr_b = persist.tile([128, NPAD], BF16, tag="scr1")
            scrs = [scr_a, scr_b]
            f1 = persist.tile([128, C, 640], BF16, tag="f1")
            f2 = persist.tile([128, C, 320], BF16, tag="f2")
            f3 = persist.tile([128, C, 160], BF16, tag="f3")
            f4 = persist.tile([128, C, 80], BF16, tag="f4")
            scr8 = persist.tile([128, T8W], BF16, tag="scr8")

            # --- main loop over row-tiles ---
            for t in range(NT):
                scr = scrs[t % 2]
                lhs_t = rows_sb[:, :, t * 128:(t + 1) * 128]
                for s in range(NSUP):
                    lo = s * SUPER
                    ps = psA.tile([128, SUPER], F32, tag="ps")
                    for j in range(SUPER // 512):
                        nc.tensor.matmul(
                            ps[:, j * 512:(j + 1) * 512],
                            lhsT=lhs_t,
                            rhs=ft_sb[:, :, lo + j * 512:lo + (j + 1) * 512],
                            start=True,
                            stop=True,
                            perf_mode=mybir.MatmulPerfMode.DoubleRow,
                        )
                    nc.scalar.activation(
                        scr[:, lo:lo + SUPER],
                        ps[:],
                        ACTF.Exp,
                        bias=bias_exp[:],
                        scale=float(A_SCALE),
                    )
                # tail: data cols 8192..8960 + Y columns
                pst = psA.tile([128, SUPER], F32, tag="ps")
                nc.tensor.matmul(
                    pst[:, 0:512],
                    lhsT=lhs_t,
                    rhs=ft_sb[:, :, 8192:8704],
                    start=True,
                    stop=True,
                    perf_mode=mybir.MatmulPerfMode.DoubleRow,
                )
                nc.tensor.matmul(
                    pst[:, 512:512 + (TAILW - 512) + 8],
                    lhsT=lhs_t,
                    rhs=ft_sb[:, :, 8704:YOFF + 8],
                    start=True,
                    stop=True,
                    perf_mode=mybir.MatmulPerfMode.DoubleRow,
                )
                nc.scalar.activation(
                    scr[:, NSUP * SUPER:NPAD],
                    pst[:, 0:TAILW],
                    ACTF.Exp,
                    bias=bias_exp[:],
                    scale=float(A_SCALE),
                )
                nc.vector.tensor_copy(praw[:, t, :], pst[:, TAILW:TAILW + 8])

                # class-segment sums: batched bf16 fold tree + one reduce
                s3 = scr[:].rearrange("p (c w) -> p c w", c=C)
                nc.vector.tensor_tensor(
                    out=f1[:], in0=s3[:, :, 0:640], in1=s3[:, :, 640:1280],
                    op=ALU.add,
                )
                nc.vector.tensor_tensor(
                    out=f2[:], in0=f1[:, :, 0:320], in1=f1[:, :, 320:640],
                    op=ALU.add,
                )
                nc.vector.tensor_tensor(
                    out=f3[:], in0=f2[:, :, 0:160], in1=f2[:, :, 160:320],
                    op=ALU.add,
                )
                nc.vector.tensor_tensor(
                    out=f4[:], in0=f3[:, :, 0:80], in1=f3[:, :, 80:160],
                    op=ALU.add,
                )
                nc.vector.reduce_sum(
                    slots_all[:, t, 0:C], f4[:], mybir.AxisListType.X
                )

            # --- shared row-tile 64: this core's column slice ---
            ps8 = psA.tile([128, SUPER], F32, tag="ps")
            for j in range(T8W // 512):
                nc.tensor.matmul(
                    ps8[:, j * 512:(j + 1) * 512],
                    lhsT=t8r_sb[:],
                    rhs=t8c_sb[:, :, j * 512:(j + 1) * 512],
                    start=True,
                    stop=True,
                    perf_mode=mybir.MatmulPerfMode.DoubleRow,
                )
            nc.scalar.activation(
                scr8[:], ps8[:, 0:T8W], ACTF.Exp,
                bias=bias_exp[:], scale=float(A_SCALE),
            )
            s83 = scr8[:].rearrange("p (c w) -> p c w", c=T8W // 512)
            nc.vector.reduce_sum(out_t[:, 2:5], s83, mybir.AxisListType.X)

            # --- epilogue: loss, threshold, partial sums ---
            stot9 = small.tile([128, NT], F32, tag="stot9")
            nc.vector.reduce_sum(stot9[:], slots_all[:], mybir.AxisListType.X)
            owns_all = small.tile([128, NT, 8], F32, tag="owns_all")
            nc.vector.tensor_tensor(
                out=owns_all[:], in0=slots_all[:], in1=oh_sb[:], op=ALU.mult
            )
            sown9 = small.tile([128, NT], F32, tag="sown9")
            nc.vector.reduce_sum(sown9[:], owns_all[:], mybir.AxisListType.X)
            negsum9 = small.tile([128, NT], F32, tag="negsum9")
            nc.vector.tensor_tensor(
                out=negsum9[:], in0=stot9[:], in1=sown9[:], op=ALU.subtract
            )
            pm_all = small.tile([128, NT, 8], F32, tag="pm_all")
            nc.vector.tensor_tensor(
                out=pm_all[:], in0=praw[:], in1=oh_sb[:], op=ALU.mult
            )
            possel9 = small.tile([128, NT], F32, tag="possel9")
            nc.vector.reduce_sum(possel9[:], pm_all[:], mybir.AxisListType.X)

            alpha9 = meta_sb[:, 0:NT]
            beta9 = meta_sb[:, NT:2 * NT]
            valid9 = meta_sb[:, 2 * NT:3 * NT]
            pos9 = small.tile([128, NT], F32, tag="pos9")
            nc.vector.tensor_tensor(
                out=pos9[:], in0=possel9[:], in1=alpha9, op=ALU.mult
            )
            nc.vector.tensor_tensor(out=pos9[:], in0=pos9[:], in1=beta9, op=ALU.add)
            neg9 = small.tile([128, NT], F32, tag="neg9")
            nc.scalar.activation(
                neg9[:], negsum9[:], ACTF.Ln, bias=bias_ln[:], scale=1.0
            )
            loss9 = small.tile([128, NT], F32, tag="loss9")
            nc.vector.tensor_tensor(
                out=loss9[:], in0=neg9[:], in1=pos9[:], op=ALU.subtract
            )
            gt9 = small.tile([128, NT], F32, tag="gt9")
            nc.vector.tensor_scalar(
                out=gt9[:], in0=loss9[:], scalar1=0.0, scalar2=None, op0=ALU.is_gt
            )
            nc.vector.tensor_tensor(out=gt9[:], in0=gt9[:], in1=valid9, op=ALU.mult)
            contrib9 = small.tile([128, NT], F32, tag="contrib9")
            nc.vector.tensor_tensor(
                out=contrib9[:], in0=loss9[:], in1=gt9[:], op=ALU.mult
            )
            nc.vector.reduce_sum(out_t[:, 0:1], contrib9[:], mybir.AxisListType.X)
            nc.vector.reduce_sum(out_t[:, 1:2], gt9[:], mybir.AxisListType.X)
            nc.sync.dma_start(out=out_d[:], in_=out_t[:])
    return nc


def _combine(results, host):
    """Host-side unshard: merge per-core partials + finish row-tile 64."""
    ls = host["ls"]
    fs, Y = host["fs"], host["Y"]
    loss_sum = 0.0
    cnt_sum = 0.0
    for r in results:
        o = np.asarray(r["out"], dtype=np.float64)
        loss_sum += o[:, 0].sum()
        cnt_sum += o[:, 1].sum()

    # row-tile 64: rows 8192..8198 — class sums from per-core chunk sums
    pad_exp = float(
        ml_dtypes.bfloat16(np.exp(np.float32(BIAS)))
    )  # a zero pad column's exp as the device computes it
    n7 = N - B  # 7
    n_cpc = T8W // 512
    classsum = np.zeros((n7, C), dtype=np.float64)
    for core in range(NCORES):
        o = np.asarray(results[core]["out"], dtype=np.float64)
        for j in range(n_cpc):
            cls, n_pad = host["t8meta"][core * n_cpc + j]
            if cls < 0:
                continue
            classsum[:, cls] += o[:n7, 2 + j] - n_pad * pad_exp
    stot = classsum.sum(1)
    rows_ls = ls[B:N]
    sown = classsum[np.arange(n7), rows_ls]
    neg = np.log(stot - sown + EPS)
    pos_sel = np.einsum("id,di->i", fs[B:N], Y[:, rows_ls])
    selfsim = host["selfsim"][B:N]
    cnt = host["cnt"][B:N]
    pos = (A_SCALE * (pos_sel - selfsim) + BIAS * cnt) / (cnt + EPS)
    loss64 = -pos + neg
    m = loss64 > 0
    loss_sum += loss64[m].sum()
    cnt_sum += m.sum()

    val = loss_sum / max(cnt_sum, 1.0) if cnt_sum > 0 else 0.0
    return np.float32(val)


def _run(features, labels, prototypes, momentums, trace=False, trace_kwargs=None):
    per_core, host = _host_prep(features, labels, prototypes, momentums)
    nc = _build_graph()
    _split_multi_waits(nc)
    in_maps = [per_core[i] for i in range(NCORES)]
    kw = {}
    if trace:
        kw = dict(trace=True, trace_cores=list(range(NCORES)))
        if trace_kwargs:
            kw["trace_kwargs"] = trace_kwargs
    res = run_bass_kernel_spmd(nc, in_maps, core_ids=list(range(NCORES)), **kw)
    return _combine(res.results, host), res


def kernel(features, labels, prototypes, momentums):
    val, _ = _run(features, labels, prototypes, momentums)
    return np.array(val, dtype=np.float32)
